# revision 8
# baseline (speedup 1.0000x reference)
"""2-layer GCN + FC on 8 trn2 NeuronCores.

Strategy (graph/data parallel, nodes sharded 12500/core by dst):
  out = relu(dinv_d * (sum_{e:dst=d} h_scaled[src_e] + Z_d)) per GCN layer,
  where h_scaled[n] = (x @ W) * dinv[n] and Z = h_scaled + bias * sqrt(deg).
  All edge norms fold into node scaling -> no per-edge multiplies.

Per core: compute h_scaled for own 12.5k nodes, AllGather the full table,
then per 128-dst-node block: dma_gather the h_scaled rows of incoming edges
(int16 indices, table split into 4 chunks of <=32768 rows), build dst
one-hots for a whole superblock with ONE broadcast tensor_tensor, and
matmul-accumulate the segment sum in PSUM (TensorE does the scatter-add).
All per-block epilogue DMAs and vector ops are batched per superblock.
"""
import numpy as np
import ml_dtypes

import concourse.bass as bass
import concourse.bacc as bacc
import concourse.mybir as mybir
import concourse.tile as tile
from concourse.bass_utils import run_bass_kernel_spmd
from concourse.masks import make_identity

NCORES = 8
N = 100000
NSHARD = N // NCORES          # 12500
P = 128
F = 128
FOUT = 64
NBLK = (NSHARD + P - 1) // P  # 98
NPAD = NBLK * P               # 12544
LAST_ROWS = NSHARD - (NBLK - 1) * P  # 84
CHUNKS = [0, 32768, 65536, 98304, N]
NCHUNK = 4
SBSIZE = 4                    # dst blocks per superblock (1 PSUM bank)
MAXCALL_SLABS = 8             # 1024 indices per dma_gather (ring limit)

FP = mybir.dt.float32
BF = mybir.dt.bfloat16
I16 = mybir.dt.int16
I32 = mybir.dt.int32
NQ = 4  # SWDGE queues for gather issue overlap


def _preprocess(edge_index):
    src = np.asarray(edge_index[0], dtype=np.int64)
    dst = np.asarray(edge_index[1], dtype=np.int64)
    deg = (np.bincount(dst, minlength=N) + 1.0).astype(np.float32)
    dinv = (1.0 / np.sqrt(deg)).astype(np.float32)
    sd = np.sqrt(deg).astype(np.float32)

    core = dst // NSHARD
    blk = (dst % NSHARD) // P
    chunkid = np.searchsorted(CHUNKS, src, side="right") - 1
    key = (core * NBLK + blk) * NCHUNK + chunkid
    order = np.argsort(key, kind="stable")
    src_s = src[order]
    dst_s = dst[order]
    cnt = np.bincount(key, minlength=NCORES * NBLK * NCHUNK).reshape(
        NCORES, NBLK, NCHUNK
    )
    assert (cnt.sum(axis=2) > 0).all(), "empty dst block"
    slabs_bk = np.ceil(cnt.max(axis=0) / P).astype(np.int64)  # [NBLK, NCHUNK]

    # global slab order: for sb, for k, for b in sb, slabs
    sb_starts = list(range(0, NBLK, SBSIZE))
    slab_block = []   # global slab idx -> block
    slab_chunk = []
    sb_ranges = []    # per sb: (slab_start, slab_end, blocks)
    calls = []        # (sb_idx, chunk, slab_start_global, n_slabs)
    s_g = 0
    for sbi, b0 in enumerate(sb_starts):
        blocks = list(range(b0, min(b0 + SBSIZE, NBLK)))
        sb_s = s_g
        for k in range(NCHUNK):
            grp_start = s_g
            for b in blocks:
                for _ in range(slabs_bk[b, k]):
                    slab_block.append(b)
                    slab_chunk.append(k)
                    s_g += 1
            # split this (sb, k) group into calls of <= MAXCALL_SLABS slabs
            pos = grp_start
            while pos < s_g:
                ns = min(MAXCALL_SLABS, s_g - pos)
                calls.append((sbi, k, pos, ns))
                pos += ns
        sb_ranges.append((sb_s, s_g, blocks))
    S = s_g

    # per-core slab payloads
    gidx_maps = []
    dstl_maps = []
    starts = np.zeros(NCORES * NBLK * NCHUNK + 1, dtype=np.int64)
    np.cumsum(cnt.reshape(-1), out=starts[1:])
    for c in range(NCORES):
        src16 = np.zeros((S, P), dtype=np.int16)
        dstl = np.full((S, P), -1.0, dtype=np.float32)
        s_g = 0
        for sbi, b0 in enumerate(sb_starts):
            blocks = list(range(b0, min(b0 + SBSIZE, NBLK)))
            for k in range(NCHUNK):
                for b in blocks:
                    nsl = int(slabs_bk[b, k])
                    if nsl == 0:
                        continue
                    kk = (c * NBLK + b) * NCHUNK + k
                    lo, hi = starts[kk], starts[kk + 1]
                    n = hi - lo
                    es = src_s[lo:hi] - CHUNKS[k]
                    ed = (dst_s[lo:hi] % NSHARD) - b * P
                    buf_s = np.zeros(nsl * P, dtype=np.int16)
                    buf_d = np.full(nsl * P, -1.0, dtype=np.float32)
                    buf_s[:n] = es.astype(np.int16)
                    buf_d[:n] = ed.astype(np.float32)
                    src16[s_g:s_g + nsl] = buf_s.reshape(nsl, P)
                    dstl[s_g:s_g + nsl] = buf_d.reshape(nsl, P)
                    s_g += nsl
        # idx wire layout: position i in a call -> [i%16, i//16]; per slab s,
        # pos p: gidx[p%16, s*8 + p//16]; replicate x8 across partitions.
        a = src16.reshape(S, 8, 16).transpose(2, 0, 1).reshape(16, S * 8)
        gidx_maps.append(np.tile(a, (8, 1)).copy())
        dstl_maps.append(dstl.T.astype(ml_dtypes.bfloat16))  # [P, S]

    return dict(
        deg=deg, dinv=dinv, sd=sd, slabs_bk=slabs_bk, S=S,
        slab_block=slab_block, slab_chunk=slab_chunk,
        sb_ranges=sb_ranges, calls=calls,
        gidx=gidx_maps, dstl=dstl_maps,
    )


def _ap3(ap, dims):
    """Reshape a 2-D AP into 3 dims [part, mid, inner] given dims list."""
    return bass.AP(ap.tensor, ap.offset, [ap.ap[0]] + dims)


def _build(meta):
    S = meta["S"]
    sb_ranges = meta["sb_ranges"]
    calls = meta["calls"]
    slab_block = meta["slab_block"]
    max_sb_slabs = max(e - s for s, e, _ in sb_ranges)

    nc = bacc.Bacc("TRN2", target_bir_lowering=False, debug=False,
                   num_devices=NCORES, num_swdge_queues=NQ)
    xT = nc.declare_dram_parameter("xT", [P, NPAD], BF, isOutput=False)
    w1 = nc.declare_dram_parameter("w1", [F, F], BF, isOutput=False)
    w2 = nc.declare_dram_parameter("w2", [F, F], BF, isOutput=False)
    wfc = nc.declare_dram_parameter("wfc", [F, FOUT], BF, isOutput=False)
    b1r = nc.declare_dram_parameter("b1r", [P, F], FP, isOutput=False)
    b2r = nc.declare_dram_parameter("b2r", [P, F], FP, isOutput=False)
    bfcr = nc.declare_dram_parameter("bfcr", [P, FOUT], FP, isOutput=False)
    dinv_p = nc.declare_dram_parameter("dinv", [P, NBLK], FP, isOutput=False)
    sd_p = nc.declare_dram_parameter("sd", [P, NBLK], FP, isOutput=False)
    gidx_p = nc.declare_dram_parameter("gidx", [P, S * 8], I16, isOutput=False)
    dstl_p = nc.declare_dram_parameter("dstl", [P, S], BF, isOutput=False)
    out_p = nc.declare_dram_parameter("out", [NSHARD, FOUT], FP, isOutput=True)

    with tile.TileContext(nc) as tc:
        with (
            tc.tile_pool(name="const", bufs=1) as cp,
            tc.tile_pool(name="xt", bufs=3) as xp,
            tc.tile_pool(name="hs", bufs=3) as hp,
            tc.tile_pool(name="z", bufs=4) as zp,
            tc.tile_pool(name="gbuf", bufs=2) as gp,
            tc.tile_pool(name="ohsb", bufs=2) as ohp,
            tc.tile_pool(name="rt", bufs=2) as rp,
            tc.tile_pool(name="relu", bufs=3) as lp,
            tc.tile_pool(name="outp", bufs=2) as op_,
            tc.tile_pool(name="pagg", bufs=2, space="PSUM") as pag,
            tc.tile_pool(name="pmisc", bufs=2, space="PSUM") as pms,
            tc.tile_pool(name="dram", bufs=1, space="DRAM") as dr,
        ):
            # ---- constants ----
            w1_t = cp.tile([F, F], BF); nc.sync.dma_start(out=w1_t[:], in_=w1[:])
            w2_t = cp.tile([F, F], BF); nc.sync.dma_start(out=w2_t[:], in_=w2[:])
            wfc_t = cp.tile([F, FOUT], BF); nc.sync.dma_start(out=wfc_t[:], in_=wfc[:])
            b1_t = cp.tile([P, F], FP); nc.sync.dma_start(out=b1_t[:], in_=b1r[:])
            b2_t = cp.tile([P, F], FP); nc.sync.dma_start(out=b2_t[:], in_=b2r[:])
            bfc_t = cp.tile([P, FOUT], FP); nc.sync.dma_start(out=bfc_t[:], in_=bfcr[:])
            dinv_t = cp.tile([P, NBLK], FP); nc.sync.dma_start(out=dinv_t[:], in_=dinv_p[:])
            sd_t = cp.tile([P, NBLK], FP); nc.sync.dma_start(out=sd_t[:], in_=sd_p[:])
            gidx_t = cp.tile([P, S * 8], I16); nc.sync.dma_start(out=gidx_t[:], in_=gidx_p[:])
            dstl_t = cp.tile([P, S], BF); nc.sync.dma_start(out=dstl_t[:], in_=dstl_p[:])
            iota_i = cp.tile([P, F], I32)
            nc.gpsimd.iota(iota_i[:], pattern=[[1, F]], base=0, channel_multiplier=0)
            iota_f = cp.tile([P, F], BF)
            nc.vector.tensor_copy(out=iota_f[:], in_=iota_i[:])
            ident = cp.tile([P, P], BF)
            make_identity(nc, ident[:])

            # ---- internal DRAM ----
            h1_shard = dr.tile([NSHARD, F], BF)
            h2_shard = dr.tile([NSHARD, F], BF)
            # z kept block-major: [128, NBLK*F]
            z1_d = dr.tile([P, NBLK * F], FP)
            z2_d = dr.tile([P, NBLK * F], FP)
            h1_full = dr.tile([N, F], BF, addr_space="Shared")
            h2_full = dr.tile([N, F], BF, addr_space="Shared")

            def dinv_bc(b0, nblk, width=F):
                sl = dinv_t[:, b0:b0 + nblk]
                return _ap3(sl, [[1, nblk], [0, width]])

            def sd_bc(b0, nblk):
                sl = sd_t[:, b0:b0 + nblk]
                return _ap3(sl, [[1, nblk], [0, F]])

            def rep_bc(t, nblk, width):
                return _ap3(t[:], [[0, nblk], [1, width]])

            def wide(t, nblk, width):
                return _ap3(t[:, :nblk * width], [[width, nblk], [1, width]])

            def store_rows(dram_t, sbuf_w, b0, nblk, width):
                """sbuf [128, nblk*width] -> dram rows b0*128 .. (node-major)."""
                full = nblk
                if (b0 + nblk) * P > NSHARD:
                    full = nblk - 1
                if full > 0:
                    sl = dram_t[b0 * P:(b0 + full) * P, :]
                    out_ap = bass.AP(
                        sl.tensor, sl.offset,
                        [[width, P], [P * width, full], [1, width]])
                    nc.sync.dma_start(out=out_ap, in_=wide(sbuf_w, full, width))
                if full < nblk:
                    b = b0 + nblk - 1
                    nc.sync.dma_start(
                        out=dram_t[b * P:b * P + LAST_ROWS, :],
                        in_=sbuf_w[:LAST_ROWS, (nblk - 1) * width:nblk * width])

            def compute_xw(b0, nblk, lhsT_w, w_t, b_t, h_shard, z_d):
                """Batched h_scaled + Z for one superblock of own-shard nodes."""
                ps_w = pms.tile([P, SBSIZE * F], FP, space="PSUM", tag="psw")
                for bi in range(nblk):
                    nc.tensor.matmul(
                        ps_w[:, bi * F:(bi + 1) * F],
                        lhsT=lhsT_w[:, bi * F:(bi + 1) * F], rhs=w_t[:],
                        start=True, stop=True)
                hs_w = hp.tile([P, SBSIZE * F], BF)
                nc.vector.tensor_tensor(
                    out=wide(hs_w, nblk, F), in0=wide(ps_w, nblk, F),
                    in1=dinv_bc(b0, nblk), op=mybir.AluOpType.mult)
                store_rows(h_shard, hs_w, b0, nblk, F)
                zt_w = zp.tile([P, SBSIZE * F], FP, tag="zt")
                nc.vector.tensor_tensor(
                    out=wide(zt_w, nblk, F), in0=rep_bc(b_t, nblk, F),
                    in1=sd_bc(b0, nblk), op=mybir.AluOpType.mult)
                z2_w = zp.tile([P, SBSIZE * F], FP, tag="z2")
                nc.vector.tensor_tensor(
                    out=wide(z2_w, nblk, F), in0=wide(zt_w, nblk, F),
                    in1=wide(hs_w, nblk, F), op=mybir.AluOpType.add)
                nc.sync.dma_start(out=z_d[:, b0 * F:(b0 + nblk) * F],
                                  in_=z2_w[:, :nblk * F])

            # ---- phase A: layer-1 h_scaled for own shard ----
            for sb_s, sb_e, blocks in sb_ranges:
                b0, nblk = blocks[0], len(blocks)
                xt_w = xp.tile([P, SBSIZE * F], BF)
                nc.sync.dma_start(out=xt_w[:, :nblk * F],
                                  in_=xT[:, b0 * P:(b0 + nblk) * P])
                compute_xw(b0, nblk, xt_w, w1_t, b1_t, h1_shard, z1_d)

            # ---- AllGather 1 ----
            nc.gpsimd.collective_compute(
                "AllGather", mybir.AluOpType.bypass,
                replica_groups=[list(range(NCORES))],
                ins=[h1_shard.opt()], outs=[h1_full.opt()])

            # global gather-call counter: keeps queue_num aligned with the
            # tile scheduler's global SWDGE sem-lane rotation (mod 8) across
            # both agg layers — a per-layer reset desyncs sem<->queue binding
            # when a layer's call count isn't a multiple of NQ.
            qrr = [0]

            def agg_layer(table, z_d, layer):
                """Edge aggregation; per superblock produce rT (transposed relu).

                layer==1: feed L2 compute (h2_shard/z2_d); layer==2: FC out.
                """
                for sbi, (sb_s, sb_e, blocks) in enumerate(sb_ranges):
                    nsl_sb = sb_e - sb_s
                    b0, nblk = blocks[0], len(blocks)
                    gb = gp.tile([P, max_sb_slabs * F], BF, tag="gb")
                    for (csbi, k, s0, ns) in calls:
                        if csbi != sbi:
                            continue
                        o = s0 - sb_s
                        sl = gb[:, o * F:(o + ns) * F]
                        out_ap = bass.AP(sl.tensor, sl.offset,
                                         [sl.ap[0], [F, ns], [1, F]])
                        nc.gpsimd.dma_gather(
                            out_ap=out_ap,
                            in_ap=table[CHUNKS[k]:CHUNKS[k + 1], :],
                            idxs_ap=gidx_t[:, s0 * 8:(s0 + ns) * 8],
                            num_idxs=ns * P,
                            num_idxs_reg=ns * P,
                            elem_size=F,
                            single_packet=False,
                            queue_num=qrr[0] % NQ,
                        )
                        qrr[0] += 1
                    # one-hot for the whole superblock in one broadcast op
                    oh_sb = ohp.tile([P, max_sb_slabs * F], BF, tag="oh")
                    dsl = dstl_t[:, sb_s:sb_e]
                    nc.vector.tensor_tensor(
                        out=_ap3(oh_sb[:, :nsl_sb * F], [[F, nsl_sb], [1, F]]),
                        in0=_ap3(iota_f[:], [[0, nsl_sb], [1, F]]),
                        in1=_ap3(dsl, [[1, nsl_sb], [0, F]]),
                        op=mybir.AluOpType.is_equal)
                    psum = pag.tile([P, SBSIZE * F], FP, space="PSUM", tag="pa")
                    for bi, b in enumerate(blocks):
                        slabs = [s for s in range(sb_s, sb_e) if slab_block[s] == b]
                        for si, s in enumerate(slabs):
                            o = s - sb_s
                            nc.tensor.matmul(
                                psum[:, bi * F:(bi + 1) * F],
                                lhsT=oh_sb[:, o * F:(o + 1) * F],
                                rhs=gb[:, o * F:(o + 1) * F],
                                start=(si == 0), stop=(si == len(slabs) - 1))
                    # batched epilogue for the superblock
                    z_w = zp.tile([P, SBSIZE * F], FP, tag="ze")
                    nc.sync.dma_start(out=z_w[:, :nblk * F],
                                      in_=z_d[:, b0 * F:(b0 + nblk) * F])
                    t_w = lp.tile([P, SBSIZE * F], FP, tag="t1")
                    nc.vector.tensor_tensor(
                        out=wide(t_w, nblk, F), in0=wide(psum, nblk, F),
                        in1=wide(z_w, nblk, F), op=mybir.AluOpType.add)
                    rm_w = lp.tile([P, SBSIZE * F], FP, tag="rm")
                    nc.vector.tensor_tensor(
                        out=wide(rm_w, nblk, F), in0=wide(t_w, nblk, F),
                        in1=dinv_bc(b0, nblk), op=mybir.AluOpType.mult)
                    r_w = lp.tile([P, SBSIZE * F], BF, tag="t2")
                    nc.vector.tensor_scalar(
                        out=r_w[:, :nblk * F], in0=rm_w[:, :nblk * F],
                        scalar1=0.0, scalar2=None, op0=mybir.AluOpType.max)
                    pst_w = pms.tile([P, SBSIZE * F], BF, space="PSUM", tag="pst")
                    for bi in range(nblk):
                        nc.tensor.transpose(
                            out=pst_w[:, bi * F:(bi + 1) * F],
                            in_=r_w[:, bi * F:(bi + 1) * F], identity=ident[:])
                    rT_w = rp.tile([P, SBSIZE * F], BF)
                    nc.vector.tensor_copy(out=rT_w[:, :nblk * F],
                                          in_=pst_w[:, :nblk * F])
                    if layer == 1:
                        compute_xw(b0, nblk, rT_w, w2_t, b2_t, h2_shard, z2_d)
                    else:
                        pfc_w = pms.tile([P, SBSIZE * FOUT], FP, space="PSUM",
                                         tag="pfc")
                        for bi in range(nblk):
                            nc.tensor.matmul(
                                pfc_w[:, bi * FOUT:(bi + 1) * FOUT],
                                lhsT=rT_w[:, bi * F:(bi + 1) * F], rhs=wfc_t[:],
                                start=True, stop=True)
                        ot_w = op_.tile([P, SBSIZE * FOUT], FP)
                        nc.vector.tensor_tensor(
                            out=wide(ot_w, nblk, FOUT),
                            in0=wide(pfc_w, nblk, FOUT),
                            in1=rep_bc(bfc_t, nblk, FOUT),
                            op=mybir.AluOpType.add)
                        store_rows(out_p, ot_w, b0, nblk, FOUT)

            # ---- phase C: L1 aggregation + L2 compute ----
            agg_layer(h1_full, z1_d, layer=1)

            # ---- AllGather 2 ----
            nc.gpsimd.collective_compute(
                "AllGather", mybir.AluOpType.bypass,
                replica_groups=[list(range(NCORES))],
                ins=[h2_shard.opt()], outs=[h2_full.opt()])

            # ---- phase E: L2 aggregation + FC ----
            agg_layer(h2_full, z2_d, layer=2)

    nc.compile()
    return nc


_CACHE = {}


def _get_nc(meta):
    key = meta["slabs_bk"].tobytes()
    if key not in _CACHE:
        _CACHE[key] = _build(meta)
    return _CACHE[key]


def _pack_inputs(x, W1, b1, W2, b2, Wfc, bfc, meta):
    x = np.asarray(x, dtype=np.float32)
    dinv = meta["dinv"]
    sd = meta["sd"]
    in_maps = []
    for c in range(NCORES):
        lo = c * NSHARD
        xT = np.zeros((P, NPAD), dtype=ml_dtypes.bfloat16)
        xT[:, :NSHARD] = x[lo:lo + NSHARD].T.astype(ml_dtypes.bfloat16)
        dv = np.ones(NPAD, dtype=np.float32)
        dv[:NSHARD] = dinv[lo:lo + NSHARD]
        sdv = np.ones(NPAD, dtype=np.float32)
        sdv[:NSHARD] = sd[lo:lo + NSHARD]
        in_maps.append({
            "xT": xT,
            "w1": np.asarray(W1).astype(ml_dtypes.bfloat16),
            "w2": np.asarray(W2).astype(ml_dtypes.bfloat16),
            "wfc": np.asarray(Wfc).astype(ml_dtypes.bfloat16),
            "b1r": np.tile(np.asarray(b1, dtype=np.float32)[None, :], (P, 1)),
            "b2r": np.tile(np.asarray(b2, dtype=np.float32)[None, :], (P, 1)),
            "bfcr": np.tile(np.asarray(bfc, dtype=np.float32)[None, :], (P, 1)),
            "dinv": dv.reshape(NBLK, P).T.copy(),
            "sd": sdv.reshape(NBLK, P).T.copy(),
            "gidx": meta["gidx"][c],
            "dstl": meta["dstl"][c],
        })
    return in_maps


def run(x, edge_index, W1, b1, W2, b2, Wfc, bfc, trace=False):
    meta = _preprocess(edge_index)
    nc = _get_nc(meta)
    in_maps = _pack_inputs(x, W1, b1, W2, b2, Wfc, bfc, meta)
    r = run_bass_kernel_spmd(nc, in_maps, list(range(NCORES)), trace=trace)
    out = np.concatenate([np.asarray(r.results[c]["out"]) for c in range(NCORES)], axis=0)
    return out.astype(np.float32), r


def kernel(**inputs):
    out, _ = run(**inputs)
    return out


def bench(x, edge_index, W1, b1, W2, b2, Wfc, bfc, reps=6):
    """Time repeated kernel executions with device-resident inputs.

    Returns (best_wall_ns, out) — min over reps of the jitted sharded call,
    inputs pre-staged on the 8 devices, excluding host pack/compile.
    """
    import time
    import jax
    import jax.numpy as jnp
    from jax.sharding import Mesh, PartitionSpec, NamedSharding

    meta = _preprocess(edge_index)
    nc = _get_nc(meta)
    in_maps = _pack_inputs(x, W1, b1, W2, b2, Wfc, bfc, meta)

    from concourse import bass2jax
    import concourse.mybir as mb
    bass2jax.install_neuronx_cc_hook()

    partition_name = nc.partition_id_tensor.name if nc.partition_id_tensor else None
    in_names, out_names, out_avals, zero_outs = [], [], [], []
    for alloc in nc.m.functions[0].allocations:
        if not isinstance(alloc, mb.MemoryLocationSet):
            continue
        name = alloc.memorylocations[0].name
        if alloc.kind == "ExternalInput":
            if name != partition_name:
                in_names.append(name)
        elif alloc.kind == "ExternalOutput":
            shape = tuple(alloc.tensor_shape)
            dtype = mb.dt.np(alloc.dtype)
            out_names.append(name)
            out_avals.append(jax.core.ShapedArray(shape, dtype))
            zero_outs.append(np.zeros(shape, dtype))
    n_params = len(in_names)
    n_outs = len(out_avals)
    in_names_all = in_names + out_names
    if partition_name is not None:
        in_names_all = in_names_all + [partition_name]

    def _body(*args):
        operands = list(args)
        if partition_name is not None:
            operands.append(bass2jax.partition_id_tensor())
        outs = bass2jax._bass_exec_p.bind(
            *operands,
            out_avals=tuple(out_avals),
            in_names=tuple(in_names_all),
            out_names=tuple(out_names),
            lowering_input_output_aliases=(),
            sim_require_finite=True,
            sim_require_nnan=True,
            nc=nc,
        )
        return tuple(outs)

    from jax.experimental.shard_map import shard_map
    devices = jax.devices()[:NCORES]
    mesh = Mesh(np.asarray(devices), ("core",))
    in_specs = (PartitionSpec("core"),) * (n_params + n_outs)
    out_specs = (PartitionSpec("core"),) * n_outs
    sharded = jax.jit(
        shard_map(_body, mesh=mesh, in_specs=in_specs, out_specs=out_specs,
                  check_rep=False),
        keep_unused=True,
    )
    sh = NamedSharding(mesh, PartitionSpec("core"))
    concat_in = [
        jax.device_put(
            np.concatenate([np.asarray(in_maps[c][n]) for c in range(NCORES)], 0), sh)
        for n in in_names
    ]
    concat_zeros = [
        jax.device_put(np.zeros((NCORES * z.shape[0], *z.shape[1:]), z.dtype), sh)
        for z in zero_outs
    ]
    for a in concat_in + concat_zeros:
        a.block_until_ready()

    best = None
    out_arrs = None
    for _ in range(reps):
        t0 = time.perf_counter()
        out_arrs = sharded(*concat_in, *concat_zeros)
        for o in out_arrs:
            o.block_until_ready()
        dt = time.perf_counter() - t0
        best = dt if best is None or dt < best else best
    out = np.asarray(out_arrs[out_names.index("out")]).reshape(
        NCORES, NSHARD, FOUT).reshape(N, FOUT)
    return int(best * 1e9), out


# revision 9
# speedup vs baseline: 1.1094x; 1.1094x over previous
"""2-layer GCN + FC on 8 trn2 NeuronCores.

Strategy (graph/data parallel, nodes sharded 12500/core by dst):
  out = relu(dinv_d * (sum_{e:dst=d} h_scaled[src_e] + Z_d)) per GCN layer,
  where h_scaled[n] = (x @ W) * dinv[n] and Z = h_scaled + bias * sqrt(deg).
  All edge norms fold into node scaling -> no per-edge multiplies.

Per core: compute h_scaled for own 12.5k nodes, AllGather the full table,
then per 128-dst-node block: dma_gather the h_scaled rows of incoming edges
(int16 indices, table split into 4 chunks of <=32768 rows), build dst
one-hots for a whole superblock with ONE broadcast tensor_tensor, and
matmul-accumulate the segment sum in PSUM (TensorE does the scatter-add).
All per-block epilogue DMAs and vector ops are batched per superblock.
"""
import numpy as np
import ml_dtypes

import concourse.bass as bass
import concourse.bacc as bacc
import concourse.mybir as mybir
import concourse.tile as tile
from concourse.bass_utils import run_bass_kernel_spmd
from concourse.masks import make_identity

NCORES = 8
N = 100000
NSHARD = N // NCORES          # 12500
P = 128
F = 128
FOUT = 64
NBLK = (NSHARD + P - 1) // P  # 98
NPAD = NBLK * P               # 12544
LAST_ROWS = NSHARD - (NBLK - 1) * P  # 84
CHUNKS = [0, 32768, 65536, 98304, N]
NCHUNK = 4
SBSIZE = 4                    # dst blocks per superblock (1 PSUM bank)
MAXCALL_SLABS = 8             # 1024 indices per dma_gather (ring limit)

FP = mybir.dt.float32
BF = mybir.dt.bfloat16
I16 = mybir.dt.int16
I32 = mybir.dt.int32
NQ = 4  # SWDGE queues for gather issue overlap


def _preprocess(edge_index):
    src = np.asarray(edge_index[0], dtype=np.int64)
    dst = np.asarray(edge_index[1], dtype=np.int64)
    deg = (np.bincount(dst, minlength=N) + 1.0).astype(np.float32)
    dinv = (1.0 / np.sqrt(deg)).astype(np.float32)
    sd = np.sqrt(deg).astype(np.float32)

    core = dst // NSHARD
    blk = (dst % NSHARD) // P
    chunkid = np.searchsorted(CHUNKS, src, side="right") - 1
    key = (core * NBLK + blk) * NCHUNK + chunkid
    order = np.argsort(key, kind="stable")
    src_s = src[order]
    dst_s = dst[order]
    cnt = np.bincount(key, minlength=NCORES * NBLK * NCHUNK).reshape(
        NCORES, NBLK, NCHUNK
    )
    assert (cnt.sum(axis=2) > 0).all(), "empty dst block"
    slabs_bk = np.ceil(cnt.max(axis=0) / P).astype(np.int64)  # [NBLK, NCHUNK]

    # global slab order: for sb, for k, for b in sb, slabs
    sb_starts = list(range(0, NBLK, SBSIZE))
    slab_block = []   # global slab idx -> block
    slab_chunk = []
    sb_ranges = []    # per sb: (slab_start, slab_end, blocks)
    calls = []        # (sb_idx, chunk, slab_start_global, n_slabs)
    s_g = 0
    for sbi, b0 in enumerate(sb_starts):
        blocks = list(range(b0, min(b0 + SBSIZE, NBLK)))
        sb_s = s_g
        for k in range(NCHUNK):
            grp_start = s_g
            for b in blocks:
                for _ in range(slabs_bk[b, k]):
                    slab_block.append(b)
                    slab_chunk.append(k)
                    s_g += 1
            # split this (sb, k) group into calls of <= MAXCALL_SLABS slabs
            pos = grp_start
            while pos < s_g:
                ns = min(MAXCALL_SLABS, s_g - pos)
                calls.append((sbi, k, pos, ns))
                pos += ns
        sb_ranges.append((sb_s, s_g, blocks))
    S = s_g

    # per-core slab payloads
    gidx_maps = []
    dstl_maps = []
    starts = np.zeros(NCORES * NBLK * NCHUNK + 1, dtype=np.int64)
    np.cumsum(cnt.reshape(-1), out=starts[1:])
    for c in range(NCORES):
        src16 = np.zeros((S, P), dtype=np.int16)
        dstl = np.full((S, P), -1.0, dtype=np.float32)
        s_g = 0
        for sbi, b0 in enumerate(sb_starts):
            blocks = list(range(b0, min(b0 + SBSIZE, NBLK)))
            for k in range(NCHUNK):
                for b in blocks:
                    nsl = int(slabs_bk[b, k])
                    if nsl == 0:
                        continue
                    kk = (c * NBLK + b) * NCHUNK + k
                    lo, hi = starts[kk], starts[kk + 1]
                    n = hi - lo
                    es = src_s[lo:hi] - CHUNKS[k]
                    ed = (dst_s[lo:hi] % NSHARD) - b * P
                    buf_s = np.zeros(nsl * P, dtype=np.int16)
                    buf_d = np.full(nsl * P, -1.0, dtype=np.float32)
                    buf_s[:n] = es.astype(np.int16)
                    buf_d[:n] = ed.astype(np.float32)
                    src16[s_g:s_g + nsl] = buf_s.reshape(nsl, P)
                    dstl[s_g:s_g + nsl] = buf_d.reshape(nsl, P)
                    s_g += nsl
        # idx wire layout: position i in a call -> [i%16, i//16]; per slab s,
        # pos p: gidx[p%16, s*8 + p//16]; replicate x8 across partitions.
        a = src16.reshape(S, 8, 16).transpose(2, 0, 1).reshape(16, S * 8)
        gidx_maps.append(np.tile(a, (8, 1)).copy())
        dstl_maps.append(dstl.T.astype(ml_dtypes.bfloat16))  # [P, S]

    return dict(
        deg=deg, dinv=dinv, sd=sd, slabs_bk=slabs_bk, S=S,
        slab_block=slab_block, slab_chunk=slab_chunk,
        sb_ranges=sb_ranges, calls=calls,
        gidx=gidx_maps, dstl=dstl_maps,
    )


def _ap3(ap, dims):
    """Reshape a 2-D AP into 3 dims [part, mid, inner] given dims list."""
    return bass.AP(ap.tensor, ap.offset, [ap.ap[0]] + dims)


def _build(meta):
    S = meta["S"]
    sb_ranges = meta["sb_ranges"]
    calls = meta["calls"]
    slab_block = meta["slab_block"]
    max_sb_slabs = max(e - s for s, e, _ in sb_ranges)

    nc = bacc.Bacc("TRN2", target_bir_lowering=False, debug=False,
                   num_devices=NCORES, num_swdge_queues=NQ)
    xT = nc.declare_dram_parameter("xT", [P, NPAD], BF, isOutput=False)
    w1 = nc.declare_dram_parameter("w1", [F, F], BF, isOutput=False)
    w2 = nc.declare_dram_parameter("w2", [F, F], BF, isOutput=False)
    wfc = nc.declare_dram_parameter("wfc", [F, FOUT], BF, isOutput=False)
    b1r = nc.declare_dram_parameter("b1r", [P, F], FP, isOutput=False)
    b2r = nc.declare_dram_parameter("b2r", [P, F], FP, isOutput=False)
    bfcr = nc.declare_dram_parameter("bfcr", [P, FOUT], FP, isOutput=False)
    dinv_p = nc.declare_dram_parameter("dinv", [P, NBLK], FP, isOutput=False)
    sd_p = nc.declare_dram_parameter("sd", [P, NBLK], FP, isOutput=False)
    gidx_p = nc.declare_dram_parameter("gidx", [P, S * 8], I16, isOutput=False)
    dstl_p = nc.declare_dram_parameter("dstl", [P, S], BF, isOutput=False)
    out_p = nc.declare_dram_parameter("out", [NSHARD, FOUT], FP, isOutput=True)

    with tile.TileContext(nc) as tc:
        with (
            tc.tile_pool(name="const", bufs=1) as cp,
            tc.tile_pool(name="xt", bufs=3) as xp,
            tc.tile_pool(name="hs", bufs=3) as hp,
            tc.tile_pool(name="z", bufs=4) as zp,
            tc.tile_pool(name="gbuf", bufs=2) as gp,
            tc.tile_pool(name="ohsb", bufs=2) as ohp,
            tc.tile_pool(name="rt", bufs=2) as rp,
            tc.tile_pool(name="relu", bufs=3) as lp,
            tc.tile_pool(name="outp", bufs=2) as op_,
            tc.tile_pool(name="pagg", bufs=2, space="PSUM") as pag,
            tc.tile_pool(name="pmisc", bufs=2, space="PSUM") as pms,
            tc.tile_pool(name="dram", bufs=1, space="DRAM") as dr,
        ):
            # ---- constants ----
            w1_t = cp.tile([F, F], BF); nc.sync.dma_start(out=w1_t[:], in_=w1[:])
            w2_t = cp.tile([F, F], BF); nc.sync.dma_start(out=w2_t[:], in_=w2[:])
            wfc_t = cp.tile([F, FOUT], BF); nc.sync.dma_start(out=wfc_t[:], in_=wfc[:])
            b1_t = cp.tile([P, F], FP); nc.sync.dma_start(out=b1_t[:], in_=b1r[:])
            b2_t = cp.tile([P, F], FP); nc.sync.dma_start(out=b2_t[:], in_=b2r[:])
            bfc_t = cp.tile([P, FOUT], FP); nc.sync.dma_start(out=bfc_t[:], in_=bfcr[:])
            dinv_t = cp.tile([P, NBLK], FP); nc.sync.dma_start(out=dinv_t[:], in_=dinv_p[:])
            sd_t = cp.tile([P, NBLK], FP); nc.sync.dma_start(out=sd_t[:], in_=sd_p[:])
            gidx_t = cp.tile([P, S * 8], I16); nc.sync.dma_start(out=gidx_t[:], in_=gidx_p[:])
            dstl_t = cp.tile([P, S], BF); nc.sync.dma_start(out=dstl_t[:], in_=dstl_p[:])
            iota_i = cp.tile([P, F], I32)
            nc.gpsimd.iota(iota_i[:], pattern=[[1, F]], base=0, channel_multiplier=0)
            iota_f = cp.tile([P, F], BF)
            nc.vector.tensor_copy(out=iota_f[:], in_=iota_i[:])
            ident = cp.tile([P, P], BF)
            make_identity(nc, ident[:])

            # ---- internal DRAM ----
            h1_shard = dr.tile([NSHARD, F], BF)
            h2_shard = dr.tile([NSHARD, F], BF)
            # z kept block-major: [128, NBLK*F]
            z1_d = dr.tile([P, NBLK * F], FP)
            z2_d = dr.tile([P, NBLK * F], FP)
            h1_full = dr.tile([N, F], BF, addr_space="Shared")
            h2_full = dr.tile([N, F], BF, addr_space="Shared")

            def dinv_bc(b0, nblk, width=F):
                sl = dinv_t[:, b0:b0 + nblk]
                return _ap3(sl, [[1, nblk], [0, width]])

            def sd_bc(b0, nblk):
                sl = sd_t[:, b0:b0 + nblk]
                return _ap3(sl, [[1, nblk], [0, F]])

            def rep_bc(t, nblk, width):
                return _ap3(t[:], [[0, nblk], [1, width]])

            def wide(t, nblk, width):
                return _ap3(t[:, :nblk * width], [[width, nblk], [1, width]])

            def store_rows(dram_t, sbuf_w, b0, nblk, width):
                """sbuf [128, nblk*width] -> dram rows b0*128 .. (node-major)."""
                full = nblk
                if (b0 + nblk) * P > NSHARD:
                    full = nblk - 1
                if full > 0:
                    sl = dram_t[b0 * P:(b0 + full) * P, :]
                    out_ap = bass.AP(
                        sl.tensor, sl.offset,
                        [[width, P], [P * width, full], [1, width]])
                    nc.sync.dma_start(out=out_ap, in_=wide(sbuf_w, full, width))
                if full < nblk:
                    b = b0 + nblk - 1
                    nc.sync.dma_start(
                        out=dram_t[b * P:b * P + LAST_ROWS, :],
                        in_=sbuf_w[:LAST_ROWS, (nblk - 1) * width:nblk * width])

            def compute_xw(b0, nblk, lhsT_w, w_t, b_t, h_shard, z_d):
                """Batched h_scaled + Z for one superblock of own-shard nodes."""
                ps_w = pms.tile([P, SBSIZE * F], FP, space="PSUM", tag="psw")
                for bi in range(nblk):
                    nc.tensor.matmul(
                        ps_w[:, bi * F:(bi + 1) * F],
                        lhsT=lhsT_w[:, bi * F:(bi + 1) * F], rhs=w_t[:],
                        start=True, stop=True)
                hs_w = hp.tile([P, SBSIZE * F], BF)
                nc.vector.tensor_tensor(
                    out=wide(hs_w, nblk, F), in0=wide(ps_w, nblk, F),
                    in1=dinv_bc(b0, nblk), op=mybir.AluOpType.mult)
                store_rows(h_shard, hs_w, b0, nblk, F)
                zt_w = zp.tile([P, SBSIZE * F], FP, tag="zt")
                nc.vector.tensor_tensor(
                    out=wide(zt_w, nblk, F), in0=rep_bc(b_t, nblk, F),
                    in1=sd_bc(b0, nblk), op=mybir.AluOpType.mult)
                z2_w = zp.tile([P, SBSIZE * F], FP, tag="z2")
                nc.vector.tensor_tensor(
                    out=wide(z2_w, nblk, F), in0=wide(zt_w, nblk, F),
                    in1=wide(hs_w, nblk, F), op=mybir.AluOpType.add)
                nc.sync.dma_start(out=z_d[:, b0 * F:(b0 + nblk) * F],
                                  in_=z2_w[:, :nblk * F])

            # ---- phase A: layer-1 h_scaled for own shard ----
            for sb_s, sb_e, blocks in sb_ranges:
                b0, nblk = blocks[0], len(blocks)
                xt_w = xp.tile([P, SBSIZE * F], BF)
                nc.sync.dma_start(out=xt_w[:, :nblk * F],
                                  in_=xT[:, b0 * P:(b0 + nblk) * P])
                compute_xw(b0, nblk, xt_w, w1_t, b1_t, h1_shard, z1_d)

            # ---- AllGather 1 ----
            nc.gpsimd.collective_compute(
                "AllGather", mybir.AluOpType.bypass,
                replica_groups=[list(range(NCORES))],
                ins=[h1_shard.opt()], outs=[h1_full.opt()])

            # global gather-call counter: keeps queue_num aligned with the
            # tile scheduler's global SWDGE sem-lane rotation (mod 8) across
            # both agg layers — a per-layer reset desyncs sem<->queue binding
            # when a layer's call count isn't a multiple of NQ.
            qrr = [0]

            def agg_layer(table, z_d, layer):
                """Edge aggregation; per superblock produce rT (transposed relu).

                layer==1: feed L2 compute (h2_shard/z2_d); layer==2: FC out.
                """
                for sbi, (sb_s, sb_e, blocks) in enumerate(sb_ranges):
                    nsl_sb = sb_e - sb_s
                    b0, nblk = blocks[0], len(blocks)
                    gb = gp.tile([P, max_sb_slabs * F], BF, tag="gb")
                    for (csbi, k, s0, ns) in calls:
                        if csbi != sbi:
                            continue
                        o = s0 - sb_s
                        sl = gb[:, o * F:(o + ns) * F]
                        out_ap = bass.AP(sl.tensor, sl.offset,
                                         [sl.ap[0], [F, ns], [1, F]])
                        nc.gpsimd.dma_gather(
                            out_ap=out_ap,
                            in_ap=table[CHUNKS[k]:CHUNKS[k + 1], :],
                            idxs_ap=gidx_t[:, s0 * 8:(s0 + ns) * 8],
                            num_idxs=ns * P,
                            num_idxs_reg=ns * P,
                            elem_size=F,
                            single_packet=True,
                            queue_num=qrr[0] % NQ,
                        )
                        qrr[0] += 1
                    # one-hot for the whole superblock in one broadcast op
                    oh_sb = ohp.tile([P, max_sb_slabs * F], BF, tag="oh")
                    dsl = dstl_t[:, sb_s:sb_e]
                    nc.vector.tensor_tensor(
                        out=_ap3(oh_sb[:, :nsl_sb * F], [[F, nsl_sb], [1, F]]),
                        in0=_ap3(iota_f[:], [[0, nsl_sb], [1, F]]),
                        in1=_ap3(dsl, [[1, nsl_sb], [0, F]]),
                        op=mybir.AluOpType.is_equal)
                    psum = pag.tile([P, SBSIZE * F], FP, space="PSUM", tag="pa")
                    for bi, b in enumerate(blocks):
                        slabs = [s for s in range(sb_s, sb_e) if slab_block[s] == b]
                        for si, s in enumerate(slabs):
                            o = s - sb_s
                            nc.tensor.matmul(
                                psum[:, bi * F:(bi + 1) * F],
                                lhsT=oh_sb[:, o * F:(o + 1) * F],
                                rhs=gb[:, o * F:(o + 1) * F],
                                start=(si == 0), stop=(si == len(slabs) - 1))
                    # batched epilogue for the superblock
                    z_w = zp.tile([P, SBSIZE * F], FP, tag="ze")
                    nc.sync.dma_start(out=z_w[:, :nblk * F],
                                      in_=z_d[:, b0 * F:(b0 + nblk) * F])
                    t_w = lp.tile([P, SBSIZE * F], FP, tag="t1")
                    nc.vector.tensor_tensor(
                        out=wide(t_w, nblk, F), in0=wide(psum, nblk, F),
                        in1=wide(z_w, nblk, F), op=mybir.AluOpType.add)
                    rm_w = lp.tile([P, SBSIZE * F], FP, tag="rm")
                    nc.vector.tensor_tensor(
                        out=wide(rm_w, nblk, F), in0=wide(t_w, nblk, F),
                        in1=dinv_bc(b0, nblk), op=mybir.AluOpType.mult)
                    r_w = lp.tile([P, SBSIZE * F], BF, tag="t2")
                    nc.vector.tensor_scalar(
                        out=r_w[:, :nblk * F], in0=rm_w[:, :nblk * F],
                        scalar1=0.0, scalar2=None, op0=mybir.AluOpType.max)
                    pst_w = pms.tile([P, SBSIZE * F], BF, space="PSUM", tag="pst")
                    for bi in range(nblk):
                        nc.tensor.transpose(
                            out=pst_w[:, bi * F:(bi + 1) * F],
                            in_=r_w[:, bi * F:(bi + 1) * F], identity=ident[:])
                    rT_w = rp.tile([P, SBSIZE * F], BF)
                    nc.vector.tensor_copy(out=rT_w[:, :nblk * F],
                                          in_=pst_w[:, :nblk * F])
                    if layer == 1:
                        compute_xw(b0, nblk, rT_w, w2_t, b2_t, h2_shard, z2_d)
                    else:
                        pfc_w = pms.tile([P, SBSIZE * FOUT], FP, space="PSUM",
                                         tag="pfc")
                        for bi in range(nblk):
                            nc.tensor.matmul(
                                pfc_w[:, bi * FOUT:(bi + 1) * FOUT],
                                lhsT=rT_w[:, bi * F:(bi + 1) * F], rhs=wfc_t[:],
                                start=True, stop=True)
                        ot_w = op_.tile([P, SBSIZE * FOUT], FP)
                        nc.vector.tensor_tensor(
                            out=wide(ot_w, nblk, FOUT),
                            in0=wide(pfc_w, nblk, FOUT),
                            in1=rep_bc(bfc_t, nblk, FOUT),
                            op=mybir.AluOpType.add)
                        store_rows(out_p, ot_w, b0, nblk, FOUT)

            # ---- phase C: L1 aggregation + L2 compute ----
            agg_layer(h1_full, z1_d, layer=1)

            # ---- AllGather 2 ----
            nc.gpsimd.collective_compute(
                "AllGather", mybir.AluOpType.bypass,
                replica_groups=[list(range(NCORES))],
                ins=[h2_shard.opt()], outs=[h2_full.opt()])

            # ---- phase E: L2 aggregation + FC ----
            agg_layer(h2_full, z2_d, layer=2)

    nc.compile()
    return nc


_CACHE = {}


def _get_nc(meta):
    key = meta["slabs_bk"].tobytes()
    if key not in _CACHE:
        _CACHE[key] = _build(meta)
    return _CACHE[key]


def _pack_inputs(x, W1, b1, W2, b2, Wfc, bfc, meta):
    x = np.asarray(x, dtype=np.float32)
    dinv = meta["dinv"]
    sd = meta["sd"]
    in_maps = []
    for c in range(NCORES):
        lo = c * NSHARD
        xT = np.zeros((P, NPAD), dtype=ml_dtypes.bfloat16)
        xT[:, :NSHARD] = x[lo:lo + NSHARD].T.astype(ml_dtypes.bfloat16)
        dv = np.ones(NPAD, dtype=np.float32)
        dv[:NSHARD] = dinv[lo:lo + NSHARD]
        sdv = np.ones(NPAD, dtype=np.float32)
        sdv[:NSHARD] = sd[lo:lo + NSHARD]
        in_maps.append({
            "xT": xT,
            "w1": np.asarray(W1).astype(ml_dtypes.bfloat16),
            "w2": np.asarray(W2).astype(ml_dtypes.bfloat16),
            "wfc": np.asarray(Wfc).astype(ml_dtypes.bfloat16),
            "b1r": np.tile(np.asarray(b1, dtype=np.float32)[None, :], (P, 1)),
            "b2r": np.tile(np.asarray(b2, dtype=np.float32)[None, :], (P, 1)),
            "bfcr": np.tile(np.asarray(bfc, dtype=np.float32)[None, :], (P, 1)),
            "dinv": dv.reshape(NBLK, P).T.copy(),
            "sd": sdv.reshape(NBLK, P).T.copy(),
            "gidx": meta["gidx"][c],
            "dstl": meta["dstl"][c],
        })
    return in_maps


def run(x, edge_index, W1, b1, W2, b2, Wfc, bfc, trace=False):
    meta = _preprocess(edge_index)
    nc = _get_nc(meta)
    in_maps = _pack_inputs(x, W1, b1, W2, b2, Wfc, bfc, meta)
    r = run_bass_kernel_spmd(nc, in_maps, list(range(NCORES)), trace=trace)
    out = np.concatenate([np.asarray(r.results[c]["out"]) for c in range(NCORES)], axis=0)
    return out.astype(np.float32), r


def kernel(**inputs):
    out, _ = run(**inputs)
    return out


def bench(x, edge_index, W1, b1, W2, b2, Wfc, bfc, reps=6):
    """Time repeated kernel executions with device-resident inputs.

    Returns (best_wall_ns, out) — min over reps of the jitted sharded call,
    inputs pre-staged on the 8 devices, excluding host pack/compile.
    """
    import time
    import jax
    import jax.numpy as jnp
    from jax.sharding import Mesh, PartitionSpec, NamedSharding

    meta = _preprocess(edge_index)
    nc = _get_nc(meta)
    in_maps = _pack_inputs(x, W1, b1, W2, b2, Wfc, bfc, meta)

    from concourse import bass2jax
    import concourse.mybir as mb
    bass2jax.install_neuronx_cc_hook()

    partition_name = nc.partition_id_tensor.name if nc.partition_id_tensor else None
    in_names, out_names, out_avals, zero_outs = [], [], [], []
    for alloc in nc.m.functions[0].allocations:
        if not isinstance(alloc, mb.MemoryLocationSet):
            continue
        name = alloc.memorylocations[0].name
        if alloc.kind == "ExternalInput":
            if name != partition_name:
                in_names.append(name)
        elif alloc.kind == "ExternalOutput":
            shape = tuple(alloc.tensor_shape)
            dtype = mb.dt.np(alloc.dtype)
            out_names.append(name)
            out_avals.append(jax.core.ShapedArray(shape, dtype))
            zero_outs.append(np.zeros(shape, dtype))
    n_params = len(in_names)
    n_outs = len(out_avals)
    in_names_all = in_names + out_names
    if partition_name is not None:
        in_names_all = in_names_all + [partition_name]

    def _body(*args):
        operands = list(args)
        if partition_name is not None:
            operands.append(bass2jax.partition_id_tensor())
        outs = bass2jax._bass_exec_p.bind(
            *operands,
            out_avals=tuple(out_avals),
            in_names=tuple(in_names_all),
            out_names=tuple(out_names),
            lowering_input_output_aliases=(),
            sim_require_finite=True,
            sim_require_nnan=True,
            nc=nc,
        )
        return tuple(outs)

    from jax.experimental.shard_map import shard_map
    devices = jax.devices()[:NCORES]
    mesh = Mesh(np.asarray(devices), ("core",))
    in_specs = (PartitionSpec("core"),) * (n_params + n_outs)
    out_specs = (PartitionSpec("core"),) * n_outs
    sharded = jax.jit(
        shard_map(_body, mesh=mesh, in_specs=in_specs, out_specs=out_specs,
                  check_rep=False),
        keep_unused=True,
    )
    sh = NamedSharding(mesh, PartitionSpec("core"))
    concat_in = [
        jax.device_put(
            np.concatenate([np.asarray(in_maps[c][n]) for c in range(NCORES)], 0), sh)
        for n in in_names
    ]
    concat_zeros = [
        jax.device_put(np.zeros((NCORES * z.shape[0], *z.shape[1:]), z.dtype), sh)
        for z in zero_outs
    ]
    for a in concat_in + concat_zeros:
        a.block_until_ready()

    best = None
    out_arrs = None
    for _ in range(reps):
        t0 = time.perf_counter()
        out_arrs = sharded(*concat_in, *concat_zeros)
        for o in out_arrs:
            o.block_until_ready()
        dt = time.perf_counter() - t0
        best = dt if best is None or dt < best else best
    out = np.asarray(out_arrs[out_names.index("out")]).reshape(
        NCORES, NSHARD, FOUT).reshape(N, FOUT)
    return int(best * 1e9), out
